# revision 1
# baseline (speedup 1.0000x reference)
"""Trainium2 Bass kernel for nn_CustomAttention (dense_transformer).

Data-parallel over batch: 8 images -> 8 NeuronCores. Per core, one
128x128x256 image runs through:
  qkv = x @ W_qkv               (PE, fp32r, channel-major output)
  qkv = DWConv3x3(qkv)          (PE, per-channel diag-weight matmuls)
  q,k normalized per spatial position; channel attention gram k^T q
  softmax over axis -2; out = v @ attn; y = out @ W_out

Layouts: "channel-major" tiles are [C partitions, spatial free]; the
gram contraction runs in natural [spatial, C] layout via PE transposes.
"""
import sys

sys.path.insert(0, "/opt/trn_rl_repo")

import numpy as np
import concourse.bass as bass
import concourse.bacc as bacc
import concourse.mybir as mybir
from concourse import masks
from concourse.tile import TileContext
from concourse.tile_rust import add_dep_helper
from concourse.bass_utils import run_bass_kernel_spmd

F32 = mybir.dt.float32
F32R = mybir.dt.float32r
AL = mybir.AluOpType
ACTF = mybir.ActivationFunctionType

B, H, W, C = 8, 128, 128, 256
HEADS, DH = 16, 16
S = H * W                 # 16384 spatial positions
C3 = 3 * C                # 768 qkv channels
NCH = C3 // 128           # 6 channel chunks (0,1:q  2,3:k  4,5:v)
TH = 8                    # interior rows per strip
NSTRIP = H // TH          # 16
PADW = W + 2              # 130
NWIN = S // 512           # 32 (stage B windows)

_cache = {}
_last_in_maps = None


def build_nc():
    nc = bacc.Bacc("TRN2", target_bir_lowering=False, debug=False)

    x_in = nc.dram_tensor("x", [S, C], F32, kind="ExternalInput")
    wqkv_in = nc.dram_tensor("w_qkv", [C, C3], F32, kind="ExternalInput")
    wdw_in = nc.dram_tensor("w_dw", [9, C3], F32, kind="ExternalInput")
    wout_in = nc.dram_tensor("w_out", [C, C], F32, kind="ExternalInput")
    temp_in = nc.dram_tensor("temperature", [HEADS], F32, kind="ExternalInput")
    mq_in = nc.dram_tensor("mean_q", [S], F32, kind="ExternalInput")
    vq_in = nc.dram_tensor("var_q", [S], F32, kind="ExternalInput")
    mk_in = nc.dram_tensor("mean_k", [S], F32, kind="ExternalInput")
    vk_in = nc.dram_tensor("var_k", [S], F32, kind="ExternalInput")
    y_out = nc.dram_tensor("out", [S, C], F32, kind="ExternalOutput")

    with TileContext(nc) as tc:
        with (
            tc.tile_pool(name="dram", bufs=1, space="DRAM") as dp,
            tc.tile_pool(name="const", bufs=1) as cp,
            tc.tile_pool(name="xnat", bufs=3) as xnp,
            tc.tile_pool(name="strip", bufs=2) as sp,
            tc.tile_pool(name="qkt", bufs=2) as qkp,
            tc.tile_pool(name="qnat", bufs=3) as qnp,
            tc.tile_pool(name="bwin", bufs=2) as bp,
            tc.tile_pool(name="ps512", bufs=4, space="PSUM") as ps512,
            tc.tile_pool(name="ps128", bufs=2, space="PSUM") as ps128,
            tc.tile_pool(name="psattn", bufs=1, space="PSUM") as psat,
        ):
            vT_spill = [[dp.tile([128, 1024], F32, name=f"vsp{ch}_{st}")
                         for st in range(NSTRIP)] for ch in range(2)]
            gram_dram = [dp.tile([128, C], F32, name=f"gramd{m}")
                         for m in range(2)]
            attn_dram = [dp.tile([128, DH], F32, name=f"attnd{m}")
                         for m in range(2)]
            _cache["dbg"] = {
                "vT_spill": [[t.tensor.name for t in row] for row in vT_spill],
                "gram_dram": [t.tensor.name for t in gram_dram],
                "attn_dram": [t.tensor.name for t in attn_dram]}
            # ---------------- constants ----------------
            eye = cp.tile([128, 128], F32, tag="eye")
            masks.make_identity(nc, eye[:])
            eye_r = cp.tile([128, 128], F32R, tag="eye_r")
            nc.vector.tensor_copy(eye_r[:], eye[:])

            zeros = cp.tile([128, PADW * 2], F32, tag="zeros")
            nc.vector.memset(zeros[:], 0.0)

            # W_qkv as two K-chunks [128, 768] (lhsT slices [128,128])
            wq = []
            for k in range(2):
                t = cp.tile([128, C3], F32R, tag=f"wq{k}")
                nc.sync.dma_start(t[:], wqkv_in[k * 128:(k + 1) * 128, :].bitcast(F32R))
                wq.append(t)
            # W_out as two K-chunks [128, 256]
            wo = []
            for k in range(2):
                t = cp.tile([128, C], F32R, tag=f"wo{k}")
                nc.sync.dma_start(t[:], wout_in[k * 128:(k + 1) * 128, :].bitcast(F32R))
                wo.append(t)

            # depthwise weights per chunk [128, 9] and diag lhsT tiles
            diags = []
            for m in range(NCH):
                wt = cp.tile([128, 9], F32, tag=f"wdw{m}")
                nc.sync.dma_start(
                    wt[:],
                    wdw_in[:].rearrange("t c -> c t")[m * 128:(m + 1) * 128, :])
                row = []
                for t in range(9):
                    d = cp.tile([128, 128], F32R, tag=f"diag{m}_{t}")
                    nc.vector.tensor_scalar(d[:], eye[:], wt[:, t:t + 1], None,
                                            op0=AL.mult)
                    row.append(d)
                diags.append(row)

            # temperature -> [128,1], partition (h,j) = temp[j]
            ttile = cp.tile([128, 1], F32, tag="ttile")
            for h0 in range(8):
                nc.sync.dma_start(ttile[h0 * 16:(h0 + 1) * 16, :],
                                  temp_in[:].rearrange("(t o) -> t o", o=1))

            # mean / rsqrt(var) tables: [128 (s%128), 128 (s//128)]
            def load_stats(mean_dram, var_dram):
                # natural [s//128, s%128] tiles (contiguous), PE-transpose
                mn = cp.tile([128, 128], F32, tag=f"mn_{mean_dram.name}")
                nc.sync.dma_start(mn[:], mean_dram[:].rearrange("(a b) -> a b", b=128))
                vn = cp.tile([128, 128], F32, tag=f"vn_{var_dram.name}")
                nc.sync.dma_start(vn[:], var_dram[:].rearrange("(a b) -> a b", b=128))
                mp = ps128.tile([128, 128], F32, tag="t128", name=f"mp_{mean_dram.name}")
                nc.tensor.transpose(mp[:], mn[:], eye[:])
                mt = cp.tile([128, 128], F32, tag=f"m_{mean_dram.name}")
                nc.scalar.copy(mt[:], mp[:])
                vp = ps128.tile([128, 128], F32, tag="t128", name=f"vp_{var_dram.name}")
                nc.tensor.transpose(vp[:], vn[:], eye[:])
                st = cp.tile([128, 128], F32, tag=f"s_{var_dram.name}")
                nc.scalar.sqrt(st[:], vp[:])
                rt = cp.tile([128, 128], F32, tag=f"r_{var_dram.name}")
                nc.vector.reciprocal(rt[:], st[:])
                return mt, rt

            mq_t, rq_t = load_stats(mq_in, vq_in)
            mk_t, rk_t = load_stats(mk_in, vk_in)

            # gram accumulators: gram'[(h,j) part, (h,i) free], 2 M-chunks
            attn_ps = [psat.tile([128, C], F32, tag=f"attn{m}", name=f"attn{m}")
                       for m in range(2)]

            spill_insts = {}
            # ---------------- stage A: strips ----------------
            for s in range(NSTRIP):
                lo = 1 if s == 0 else 0          # first computable pad row
                hi = 9 if s == NSTRIP - 1 else 10

                # x rows (image rows 8s+pr-1), transposed to channel-major
                xT = [sp.tile([128, 10, 128], F32R, tag=f"xT{k}", name=f"xT{k}_{s}")
                      for k in range(2)]
                for pr in range(lo, hi):
                    ir = TH * s + pr - 1
                    xn = xnp.tile([128, C], F32, tag="xn")
                    nc.sync.dma_start(xn[:], x_in[ir * 128:(ir + 1) * 128, :])
                    for k in range(2):
                        tp = ps128.tile([128, 128], F32, tag="t128")
                        nc.tensor.transpose(tp[:], xn[:, k * 128:(k + 1) * 128], eye[:])
                        nc.scalar.copy(xT[k][:, pr, :], tp[:])

                # qkv0^T in padded layout [128, 10, 130]
                pads = [sp.tile([128, 10, PADW], F32R, tag=f"pad{m}", name=f"pad{m}_{s}")
                        for m in range(NCH)]
                for m in range(NCH):
                    # zero pad columns (w=-1 and w=128)
                    nc.vector.tensor_copy(
                        pads[m][:, :, 0:1].rearrange("p a b -> p (a b)"),
                        zeros[:, 0:10])
                    nc.vector.tensor_copy(
                        pads[m][:, :, 129:130].rearrange("p a b -> p (a b)"),
                        zeros[:, 0:10])
                    if s == 0:
                        nc.vector.tensor_copy(pads[m][:, 0, :], zeros[:, 0:PADW])
                    if s == NSTRIP - 1:
                        nc.vector.tensor_copy(pads[m][:, 9, :], zeros[:, 0:PADW])
                    r = lo
                    while r < hi:
                        n = min(4, hi - r)
                        mm = ps512.tile([128, 4, 128], F32, tag="w512")
                        for k in range(2):
                            nc.tensor.matmul(
                                mm[:, 0:n, :],
                                wq[k][:, m * 128:(m + 1) * 128],
                                xT[k][:, r:r + n, :],
                                start=(k == 0), stop=(k == 1))
                        nc.vector.tensor_copy(pads[m][:, r:r + n, 1:129],
                                              mm[:, 0:n, :])
                        r += n

                # depthwise conv: out image rows 8s..8s+8 (pad rows 1..9)
                qkT = [qkp.tile([128, TH, 128], F32R, tag=f"qkT{m}", name=f"qkT{m}_{s}")
                       for m in range(4)]
                vhat = [qkp.tile([128, TH, 128], F32, tag=f"vh{m}", name=f"vh{m}_{s}")
                        for m in range(2)]
                for m in range(NCH):
                    for wi in range(2):
                        h0 = wi * 4
                        cv = ps512.tile([128, 4, 128], F32, tag="w512")
                        for t in range(9):
                            ky, kx = t // 3, t % 3
                            nc.tensor.matmul(
                                cv[:], diags[m][t][:],
                                pads[m][:, h0 + ky:h0 + ky + 4, kx:kx + 128],
                                start=(t == 0), stop=(t == 8))
                        if m < 4:
                            nc.scalar.copy(qkT[m][:, h0:h0 + 4, :], cv[:])
                        else:
                            nc.scalar.copy(vhat[m - 4][:, h0:h0 + 4, :], cv[:])

                # spill v̂^T
                for ch in range(2):
                    spill_insts[(ch, s)] = nc.sync.dma_start(
                        vT_spill[ch][s][:],
                        vhat[ch][:].rearrange("p a b -> p (a b)"))

                # q̂,k̂ back to natural layout + normalize, then gram
                for r in range(TH):
                    sc = TH * s + r
                    qn = qnp.tile([128, C], F32R, tag="qn")
                    kn = qnp.tile([128, C], F32R, tag="kn")
                    for m in range(2):
                        tq = ps128.tile([128, 128], F32, tag="t128")
                        nc.tensor.transpose(tq[:].bitcast(F32R),
                                            qkT[m][:, r, :], eye_r[:])
                        nc.vector.tensor_scalar(
                            qn[:, m * 128:(m + 1) * 128], tq[:],
                            mq_t[:, sc:sc + 1], rq_t[:, sc:sc + 1],
                            op0=AL.subtract, op1=AL.mult)
                        tk = ps128.tile([128, 128], F32, tag="t128")
                        nc.tensor.transpose(tk[:].bitcast(F32R),
                                            qkT[2 + m][:, r, :], eye_r[:])
                        nc.vector.tensor_scalar(
                            kn[:, m * 128:(m + 1) * 128], tk[:],
                            mk_t[:, sc:sc + 1], rk_t[:, sc:sc + 1],
                            op0=AL.subtract, op1=AL.mult)
                    first = (s == 0 and r == 0)
                    last = (s == NSTRIP - 1 and r == TH - 1)
                    for m in range(2):
                        nc.tensor.matmul(
                            attn_ps[m][:], qn[:, m * 128:(m + 1) * 128], kn[:],
                            start=first, stop=last)

            # ---------------- softmax interlude ----------------
            # attn_ps[m][(h,j) local, (h,i) global] ; want softmax over i
            asm = []
            attn_w = {}
            for m in range(2):
                # PSUM gram -> SBUF -> DRAM, then affine diag-block gather
                gsb = cp.tile([128, C], F32, tag=f"gsb{m}", name=f"gsb{m}")
                nc.vector.tensor_copy(gsb[:], attn_ps[m][:])
                gw = nc.sync.dma_start(gram_dram[m][:], gsb[:])
                pk = cp.tile([128, DH], F32, tag=f"pk{m}")
                gather = bass.AP(gram_dram[m].tensor, m * 128,
                                 [[16 * C + 16, 8], [C, 16], [1, 16]])
                pg = nc.sync.dma_start(pk[:], gather)
                add_dep_helper(pg.ins, gw.ins, reason="gram spill RAW")
                # temperature multiplies along j (= partition here)
                nc.vector.tensor_scalar(pk[:], pk[:], ttile[:], None, op0=AL.mult)
                mx = cp.tile([128, 1], F32, tag=f"mx{m}")
                nc.vector.tensor_reduce(mx[:], pk[:], axis=mybir.AxisListType.X,
                                        op=AL.max)
                nmx = cp.tile([128, 1], F32, tag=f"nmx{m}")
                nc.vector.tensor_scalar(nmx[:], mx[:], -1.0, None, op0=AL.mult)
                ex = cp.tile([128, DH], F32, tag=f"ex{m}")
                nc.scalar.activation(ex[:], pk[:], ACTF.Exp, bias=nmx[:], scale=1.0)
                sm = cp.tile([128, 1], F32, tag=f"sm{m}")
                nc.vector.tensor_reduce(sm[:], ex[:], axis=mybir.AxisListType.X,
                                        op=AL.add)
                rs = cp.tile([128, 1], F32, tag=f"rs{m}")
                nc.vector.reciprocal(rs[:], sm[:])
                sfm = cp.tile([128, DH], F32, tag=f"sfm{m}")
                nc.vector.tensor_scalar(sfm[:], ex[:], rs[:], None, op0=AL.mult)
                attn_w[m] = nc.sync.dma_start(attn_dram[m][:], sfm[:])
                asm.append(sfm)

            # block-diagonal apply matrices A[(h,i),(h,j)] = attn[i,j]

            amat = []
            for m in range(2):
                A = cp.tile([128, 128], F32R, tag=f"A{m}")
                nc.vector.tensor_copy(A[:], zeros[:, 0:128])
                for h0 in range(8):
                    ai = nc.sync.dma_start(
                        A[h0 * 16:(h0 + 1) * 16, h0 * 16:(h0 + 1) * 16],
                        attn_dram[m][:].rearrange("p i -> i p")
                        [:, h0 * 16:(h0 + 1) * 16].bitcast(F32R))
                    add_dep_helper(ai.ins, attn_w[m].ins, reason="attn spill RAW")
                amat.append(A)

            # ---------------- stage B: apply + out-proj ----------------
            for w in range(NWIN):
                vt = [bp.tile([128, 512], F32R, tag=f"vt{ch}", name=f"vt{ch}_{w}")
                      for ch in range(2)]
                oT = [bp.tile([128, 512], F32R, tag=f"oT{ch}", name=f"oT{ch}_{w}")
                      for ch in range(2)]
                for ch in range(2):
                    half = (w % 2) * 512
                    ld = nc.sync.dma_start(
                        vt[ch][:],
                        vT_spill[ch][w // 2][:, half:half + 512].bitcast(F32R))
                    add_dep_helper(ld.ins, spill_insts[(ch, w // 2)].ins,
                                   reason="v spill RAW")
                    op_ = ps512.tile([128, 512], F32, tag="w512")
                    nc.tensor.matmul(op_[:], amat[ch][:], vt[ch][:],
                                     start=True, stop=True)
                    nc.scalar.copy(oT[ch][:], op_[:])
                for i in range(4):
                    sc = w * 4 + i
                    yp = ps512.tile([128, C], F32, tag="w512")
                    for ch in range(2):
                        nc.tensor.matmul(yp[:], oT[ch][:, i * 128:(i + 1) * 128],
                                         wo[ch][:], start=(ch == 0), stop=(ch == 1))
                    ysb = bp.tile([128, C], F32, tag="ysb")
                    nc.scalar.copy(ysb[:], yp[:])
                    nc.sync.dma_start(y_out[sc * 128:(sc + 1) * 128, :], ysb[:])

    nc.compile()
    return nc


def _get_nc():
    if "nc" not in _cache:
        _cache["nc"] = build_nc()
    return _cache["nc"]


def kernel(x, w_qkv, w_dw, w_out, temperature, mean_q, var_q, mean_k, var_k):
    x = np.ascontiguousarray(np.asarray(x, np.float32))
    w_qkv = np.ascontiguousarray(np.asarray(w_qkv, np.float32))
    w_dw = np.ascontiguousarray(np.asarray(w_dw, np.float32).reshape(9, C3))
    w_out = np.ascontiguousarray(np.asarray(w_out, np.float32))
    temperature = np.ascontiguousarray(np.asarray(temperature, np.float32).reshape(HEADS))
    stats = [np.ascontiguousarray(np.asarray(t, np.float32).reshape(S))
             for t in (mean_q, var_q, mean_k, var_k)]

    in_maps = []
    for b in range(B):
        in_maps.append({
            "x": np.ascontiguousarray(x[b].reshape(S, C)),
            "w_qkv": w_qkv,
            "w_dw": w_dw,
            "w_out": w_out,
            "temperature": temperature,
            "mean_q": stats[0], "var_q": stats[1],
            "mean_k": stats[2], "var_k": stats[3],
        })
    global _last_in_maps
    _last_in_maps = in_maps
    nc = _get_nc()
    res = run_bass_kernel_spmd(nc, in_maps, list(range(B)))
    out = np.stack([res.results[b]["out"] for b in range(B)])
    return out.reshape(B, H, W, C)



# revision 29
# speedup vs baseline: 2.2289x; 2.2289x over previous
"""Trainium2 Bass kernel for nn_CustomAttention (dense_transformer).

Data-parallel over batch: 8 images -> 8 NeuronCores. Host pre-transposes
x to channel-major fp8 pairs; device pipeline per core:

  qkv   = x @ W_qkv          (PE, fp8 DoubleRow: K=256 in one 0.5c/row pass)
  pad   = fp8 padded qkv     (DVE/Act/GpSimd evictions, borders memset)
  q,k   dwconv 3x3           (PE, fp8 DoubleRow diag-pair matmuls: 5 passes/9 taps)
  q,k   transpose + normalize (PE bf16 transposes; normalize fused into eviction)
  gram  = qn^T @ kn          (PE bf16, PSUM accumulation over all spatial)
  softmax -> attn -> B_t = attn*w_t  (v-dwconv folded into apply)
  out^T = sum_t B_t^T @ shift_t(v0)  (PE fp8 DoubleRow pairs)
  y     = out^T^T @ W_out    (PE bf16), y bf16 -> host

Scales: W_qkv*16, dw-diag*64, wv*512; compensated in stats tables / W_out.
"""
import sys

sys.path.insert(0, "/opt/trn_rl_repo")

import numpy as np
import ml_dtypes

import concourse.bass as bass
import concourse.bacc as bacc
import concourse.mybir as mybir
from concourse import masks
from concourse.tile import TileContext
from concourse.tile_rust import add_dep_helper
from concourse.bass_utils import run_bass_kernel_spmd

F32 = mybir.dt.float32
BF16 = mybir.dt.bfloat16
FP8 = mybir.dt.float8e4
AL = mybir.AluOpType
ACTF = mybir.ActivationFunctionType
DR = mybir.MatmulPerfMode.DoubleRow

NPF8 = ml_dtypes.float8_e4m3
NPBF = ml_dtypes.bfloat16

B, H, W, C = 8, 128, 128, 256
HEADS, DH = 16, 16
S = H * W                 # 16384
C3 = 3 * C                # 768
TH = 8                    # interior rows per strip
NSTRIP = H // TH          # 16
PW = 132                  # padded row width (positions; w=-1..130)
NWIN = S // 512           # 32 stage-B windows

SW, SD, SB = 16.0, 64.0, 512.0   # host-side scale factors
PAIRS = [(0, 1), (2, 3), (4, 5), (6, 7), (8, 9)]   # tap pairs; 9 == zero tap

_cache = {}
_last_in_maps = None


def _off(t, h0):
    # element offset of tap t (ky=t//3, kx=t%3-1) for out-row-base h0
    return (h0 + t // 3) * PW + (t % 3)


def _pair_rhs(tile, pi, h0, ncols=128):
    """Raw AP [128, 2, ncols] for DoubleRow tap-pair pi at out-row h0 (3D only)."""
    ta, tb = PAIRS[pi]
    base = _off(ta, h0)
    delta = (_off(tb, h0) - base) if tb < 9 else 1
    full = tile[:]
    dims = [list(full.ap[0]), [delta, 2], [1, ncols]]
    return bass.AP(tile.tensor, full.offset + base, dims)


def build_nc():
    nc = bacc.Bacc("TRN2", target_bir_lowering=False, debug=False)

    x_in = nc.dram_tensor("xp", [128, 2, S], FP8, kind="ExternalInput")
    xb_in = nc.dram_tensor("xb", [128, 2, S], BF16, kind="ExternalInput")
    wq_in = nc.dram_tensor("wqp", [128, 2 * 512], FP8, kind="ExternalInput")
    wqv_in = nc.dram_tensor("wqv", [128, 2 * C], BF16, kind="ExternalInput")
    dwp_in = nc.dram_tensor("dwp", [20, 128, 256], FP8, kind="ExternalInput")
    wv_in = nc.dram_tensor("wvcols", [128, 20], F32, kind="ExternalInput")
    wo_in = nc.dram_tensor("wop", [128, 2 * C], BF16, kind="ExternalInput")
    st_in = nc.dram_tensor("stats", [128, 4 * 128], F32, kind="ExternalInput")
    temp_in = nc.dram_tensor("temperature", [HEADS], F32, kind="ExternalInput")
    y_out = nc.dram_tensor("out", [S, C], BF16, kind="ExternalOutput")

    with TileContext(nc) as tc:
        with (
            tc.tile_pool(name="dram", bufs=1, space="DRAM") as dp,
            tc.tile_pool(name="const", bufs=1) as cp,
            tc.tile_pool(name="padv", bufs=1) as pv,
            tc.tile_pool(name="xs", bufs=2) as xp_,
            tc.tile_pool(name="pads", bufs=2) as sp,
            tc.tile_pool(name="qkcm", bufs=2) as qp_,
            tc.tile_pool(name="qn", bufs=3) as np_,
            tc.tile_pool(name="bwin", bufs=3) as bp,
            tc.tile_pool(name="w512", bufs=4, space="PSUM") as ps512,
            tc.tile_pool(name="pst", bufs=2, space="PSUM") as psT,
            tc.tile_pool(name="psattn", bufs=1, space="PSUM") as psG,
        ):
            gram_dram = [dp.tile([128, C], F32, name=f"gramd{m}") for m in range(2)]
            attn_dram = [dp.tile([128, DH], F32, name=f"attnd{m}") for m in range(2)]

            # ---------------- constants ----------------
            eye = cp.tile([128, 128], F32, tag="eye")
            masks.make_identity(nc, eye[:])
            eye_bf = cp.tile([128, 128], BF16, tag="eyebf")
            nc.vector.tensor_copy(eye_bf[:], eye[:])

            wq = cp.tile([128, 2, 512], FP8, tag="wq")
            nc.sync.dma_start(wq[:].rearrange("p a b -> p (a b)"), wq_in[:])
            wqv = cp.tile([128, 2, C], BF16, tag="wqv")
            nc.sync.dma_start(wqv[:].rearrange("p a b -> p (a b)"), wqv_in[:])
            wo = cp.tile([128, 2, C], BF16, tag="wo")
            nc.sync.dma_start(wo[:].rearrange("p a b -> p (a b)"), wo_in[:])
            dwp = []
            for i in range(20):
                t = cp.tile([128, 2, 128], FP8, tag=f"dwp{i}")
                nc.sync.dma_start(t[:].rearrange("p a b -> p (a b)"), dwp_in[i, :, :])
                dwp.append(t)
            wvc = cp.tile([128, 20], F32, tag="wvc")
            nc.sync.dma_start(wvc[:], wv_in[:])
            stats = cp.tile([128, 4, 128], F32, tag="stats")
            nc.sync.dma_start(stats[:].rearrange("p a b -> p (a b)"), st_in[:])

            def ecopy(eng, out, in_):
                return eng.copy(out, in_) if eng is nc.scalar \
                    else eng.tensor_copy(out, in_)

            ttile = cp.tile([128, 1], F32, tag="ttile")
            for h0 in range(8):
                nc.sync.dma_start(ttile[h0 * 16:(h0 + 1) * 16, :],
                                  temp_in[:].rearrange("(t o) -> t o", o=1))

            # persistent v pad buffers (global rows 0..129; row = img row + 1)
            pad_v = [pv.tile([128, 130, PW], BF16, tag=f"pv{ch}", name=f"pv{ch}")
                     for ch in range(2)]
            v_deps = [[] for _ in range(NSTRIP)]
            v_border = []
            for ch in range(2):
                for ap in (pad_v[ch][:, 0:1, :], pad_v[ch][:, 129:130, :],
                           pad_v[ch][:, :, 0:1], pad_v[ch][:, :, 129:132]):
                    v_border.append(nc.gpsimd.memset(ap, 0.0))

            # gram accumulators
            gram_ps = [psG.tile([128, C], F32, tag=f"g{m}", name=f"g{m}")
                       for m in range(2)]

            # ---------------- stage A (software-pipelined) ----------------
            # gpsimd has no PSUM access: evictions go to vector/scalar only
            EV = [nc.vector, nc.vector, nc.scalar, nc.scalar, nc.vector, nc.scalar]
            state = {}

            def front(s):
                lo = 1 if s == 0 else 0
                hi = 9 if s == NSTRIP - 1 else 10
                r0 = TH * s + lo - 1          # first image row
                nrow = hi - lo
                xs = xp_.tile([128, 2, 1280], FP8, tag="xs", name=f"xs{s}")
                nc.sync.dma_start(xs[:, :, 0:nrow * 128],
                                  x_in[:, :, r0 * 128:(r0 + nrow) * 128])
                xb = xp_.tile([128, 2, 1280], BF16, tag="xb", name=f"xb{s}")
                nc.sync.dma_start(xb[:, :, 0:nrow * 128],
                                  xb_in[:, :, r0 * 128:(r0 + nrow) * 128])
                pads = [sp.tile([128, 10, PW], FP8, tag=f"pad{m}", name=f"pad{m}_{s}")
                        for m in range(4)]
                pdeps = [[] for _ in range(4)]
                for m in range(4):
                    pdeps[m].append(nc.gpsimd.memset(pads[m][:, :, 0:1], 0.0))
                    pdeps[m].append(nc.gpsimd.memset(pads[m][:, :, 129:132], 0.0))
                    if s == 0:
                        pdeps[m].append(nc.gpsimd.memset(pads[m][:, 0:1, :], 0.0))
                    if s == NSTRIP - 1:
                        pdeps[m].append(nc.gpsimd.memset(pads[m][:, 9:10, :], 0.0))
                # row groups of <=4
                groups = []
                r = lo
                while r < hi:
                    n = min(4, hi - r)
                    groups.append((r, n))
                    r += n
                for m in range(6):
                    for (ra, n) in groups:
                        qp = ps512.tile([128, 4, 128], F32, tag="w512")
                        if m < 4:
                            nc.tensor.matmul(
                                qp[:, 0:n, :], wq[:, :, m * 128:(m + 1) * 128],
                                xs[:, :, (ra - lo) * 128:(ra - lo + n) * 128],
                                start=True, stop=True, perf_mode=DR)
                        else:
                            for k in range(2):
                                nc.tensor.matmul(
                                    qp[:, 0:n, :],
                                    wqv[:, k, (m - 4) * 128:(m - 3) * 128],
                                    xb[:, k, (ra - lo) * 128:(ra - lo + n) * 128],
                                    start=(k == 0), stop=(k == 1))
                        if m < 4:
                            ev = ecopy(EV[m], pads[m][:, ra:ra + n, 1:129],
                                       qp[:, 0:n, :])
                            pdeps[m].append(ev)
                        else:
                            ch = m - 4
                            a, b = max(ra, 1), min(ra + n, 9)
                            if a < b:
                                ev = ecopy(
                                    EV[m],
                                    pad_v[ch][:, TH * s + a:TH * s + b, 1:129],
                                    qp[:, a - ra:b - ra, :])
                                v_deps[s].append(ev)
                state[s] = (pads, pdeps)

            def dwstage(s):
                pads, pdeps = state[s]
                qkcm = [qp_.tile([128, TH, 128], BF16, tag=f"cm{m}",
                                 name=f"cm{m}_{s}") for m in range(4)]
                for mc in range(4):
                    for win in range(2):
                        h0 = 4 * win
                        dpp = ps512.tile([128, 4, 128], F32, tag="w512")
                        for r in range(4):
                            for pi in range(5):
                                mm = nc.tensor.matmul(
                                    dpp[:, r, :], dwp[mc * 5 + pi][:],
                                    _pair_rhs(pads[mc], pi, h0 + r),
                                    start=(pi == 0), stop=(pi == 4),
                                    perf_mode=DR)
                                for d in pdeps[mc]:
                                    add_dep_helper(mm.ins, d.ins, reason="pad RAW")
                        ecopy(EV[2 * (mc % 2)],
                              qkcm[mc][:, h0:h0 + 4, :], dpp[:])
                state[s] = state[s] + (qkcm,)

            def tgstage(s):
                qkcm = state.pop(s)[2]
                for r in range(TH):
                    ti = TH * s + r
                    tqk = psT.tile([128, 512], BF16, tag="tqk")
                    for m in range(2):
                        nc.tensor.transpose(tqk[:, m * 128:(m + 1) * 128],
                                            qkcm[m][:, r, :], eye_bf[:])
                        nc.tensor.transpose(tqk[:, 256 + m * 128:256 + (m + 1) * 128],
                                            qkcm[2 + m][:, r, :], eye_bf[:])
                    qn = np_.tile([128, C], BF16, tag="qn")
                    kn = np_.tile([128, C], BF16, tag="kn")
                    nc.vector.tensor_scalar(qn[:], tqk[:, 0:256],
                                            stats[:, 0, ti:ti + 1],
                                            stats[:, 1, ti:ti + 1],
                                            op0=AL.mult, op1=AL.add)
                    nc.scalar.activation(kn[:], tqk[:, 256:512], ACTF.Identity,
                                         bias=stats[:, 3, ti:ti + 1],
                                         scale=stats[:, 2, ti:ti + 1])
                    first, last = ti == 0, ti == S // 128 - 1
                    for m in range(2):
                        nc.tensor.matmul(gram_ps[m][:],
                                         qn[:, m * 128:(m + 1) * 128], kn[:],
                                         start=first, stop=last)

            for s in range(NSTRIP + 2):
                if s < NSTRIP:
                    front(s)
                if 1 <= s <= NSTRIP:
                    dwstage(s - 1)
                if 2 <= s:
                    tgstage(s - 2)

            # ---------------- softmax interlude ----------------
            attn_w = {}
            for m in range(2):
                gsb = cp.tile([128, C], F32, tag=f"gsb{m}")
                nc.vector.tensor_copy(gsb[:], gram_ps[m][:])
                gw = nc.sync.dma_start(gram_dram[m][:], gsb[:])
                pk = cp.tile([128, DH], F32, tag=f"pk{m}")
                gather = bass.AP(gram_dram[m].tensor, m * 128,
                                 [[16 * C + 16, 8], [C, 16], [1, 16]])
                pg = nc.sync.dma_start(pk[:], gather)
                add_dep_helper(pg.ins, gw.ins, reason="gram spill RAW")
                nc.vector.tensor_scalar(pk[:], pk[:], ttile[:], None, op0=AL.mult)
                mx = cp.tile([128, 1], F32, tag=f"mx{m}")
                nc.vector.tensor_reduce(mx[:], pk[:], axis=mybir.AxisListType.X,
                                        op=AL.max)
                nmx = cp.tile([128, 1], F32, tag=f"nmx{m}")
                nc.vector.tensor_scalar(nmx[:], mx[:], -1.0, None, op0=AL.mult)
                ex = cp.tile([128, DH], F32, tag=f"ex{m}")
                nc.scalar.activation(ex[:], pk[:], ACTF.Exp, bias=nmx[:], scale=1.0)
                sm = cp.tile([128, 1], F32, tag=f"sm{m}")
                nc.vector.tensor_reduce(sm[:], ex[:], axis=mybir.AxisListType.X,
                                        op=AL.add)
                rs = cp.tile([128, 1], F32, tag=f"rs{m}")
                nc.vector.reciprocal(rs[:], sm[:])
                sfm = cp.tile([128, DH], F32, tag=f"sfm{m}")
                nc.vector.tensor_scalar(sfm[:], ex[:], rs[:], None, op0=AL.mult)
                attn_w[m] = nc.sync.dma_start(attn_dram[m][:], sfm[:])

            # block-diagonal A, then bf16 B_t tiles (v dwconv folded into apply)
            bpt = []
            for ch in range(2):
                A = cp.tile([128, 128], F32, tag=f"A{ch}")
                nc.vector.memset(A[:], 0.0)
                for h0 in range(8):
                    ai = nc.sync.dma_start(
                        A[h0 * 16:(h0 + 1) * 16, h0 * 16:(h0 + 1) * 16],
                        attn_dram[ch][:].rearrange("p i -> i p")
                        [:, h0 * 16:(h0 + 1) * 16])
                    add_dep_helper(ai.ins, attn_w[ch].ins, reason="attn spill RAW")
                for t in range(9):
                    bt = cp.tile([128, 128], BF16, tag=f"bp{ch}_{t}")
                    nc.vector.tensor_scalar(bt[:], A[:],
                                            wvc[:, 10 * ch + t:10 * ch + t + 1],
                                            None, op0=AL.mult)
                    bpt.append(bt)

            # ---------------- stage B: apply(+v dwconv) + out-proj ----------------
            bstate = {}

            def apply_stage(w):
                oT = [bp.tile([128, 512], BF16, tag=f"oT{ch}", name=f"oT{ch}_{w}")
                      for ch in range(2)]
                s_lo = max(0, (4 * w - 1) // TH)
                s_hi = min(NSTRIP - 1, (4 * w + 4) // TH)
                for ch in range(2):
                    op_ = ps512.tile([128, 4, 128], F32, tag="w512")
                    for t in range(9):
                        ky, kxo = t // 3, t % 3
                        mm = nc.tensor.matmul(
                            op_[:], bpt[ch * 9 + t][:],
                            pad_v[ch][:, 4 * w + ky:4 * w + ky + 4,
                                       kxo:kxo + 128],
                            start=(t == 0), stop=(t == 8))
                        for st in range(s_lo, s_hi + 1):
                            for d in v_deps[st]:
                                add_dep_helper(mm.ins, d.ins, reason="vpad RAW")
                        for d in v_border:
                            add_dep_helper(mm.ins, d.ins, reason="vpad border")
                    ecopy(EV[2 * ch], oT[ch][:], op_[:])
                bstate[w] = oT

            def proj_stage(w):
                oT = bstate.pop(w)
                for i in range(4):
                    ti = 4 * w + i
                    if i % 2 == 0:
                        ypt = ps512.tile([128, 4, 128], F32, tag="w512")
                    yp = ypt[:, 2 * (i % 2):2 * (i % 2) + 2, :]
                    for ch in range(2):
                        nc.tensor.matmul(yp, oT[ch][:, i * 128:(i + 1) * 128],
                                         wo[:, ch, :], start=(ch == 0),
                                         stop=(ch == 1))
                    ysb = bp.tile([128, C], BF16, tag="ysb")
                    ecopy(EV[2 * (i % 2)], ysb[:], yp)
                    nc.sync.dma_start(y_out[ti * 128:(ti + 1) * 128, :], ysb[:])

            for w in range(NWIN + 1):
                if w < NWIN:
                    apply_stage(w)
                if 1 <= w:
                    proj_stage(w - 1)

    nc.compile()
    return nc


def _get_nc():
    if "nc" not in _cache:
        _cache["nc"] = build_nc()
    return _cache["nc"]


def _host_prep(x, w_qkv, w_dw, w_out, temperature, mean_q, var_q, mean_k, var_k):
    x = np.asarray(x, np.float32)
    w_qkv = np.asarray(w_qkv, np.float32)
    w9 = np.asarray(w_dw, np.float32).reshape(9, C3)
    w_out = np.asarray(w_out, np.float32)
    temp = np.ascontiguousarray(np.asarray(temperature, np.float32).reshape(HEADS))
    mq, vq, mk, vk = (np.asarray(t, np.float32).reshape(S)
                      for t in (mean_q, var_q, mean_k, var_k))

    wqP = np.ascontiguousarray(
        (w_qkv[:, 0:512] * SW).reshape(2, 128, 512).transpose(1, 0, 2)).astype(NPF8)
    wqP = wqP.reshape(128, 2 * 512)
    wqV = np.ascontiguousarray(
        w_qkv[:, 512:].reshape(2, 128, C).transpose(1, 0, 2)).astype(NPBF)
    wqV = wqV.reshape(128, 2 * C)

    dwp = np.zeros((20, 128, 2, 128), np.float32)
    idx = np.arange(128)
    for mc in range(4):
        for pi, (ta, tb) in enumerate(PAIRS):
            for sub, t in enumerate((ta, tb)):
                if t < 9:
                    dwp[mc * 5 + pi, idx, sub, idx] = w9[t, mc * 128 + idx] * SD
    dwp = np.ascontiguousarray(dwp).astype(NPF8).reshape(20, 128, 256)

    wvc = np.zeros((128, 20), np.float32)
    for ch in range(2):
        for t in range(9):
            wvc[:, 10 * ch + t] = w9[t, 512 + ch * 128 + idx]

    woP = np.ascontiguousarray(
        w_out.reshape(2, 128, C).transpose(1, 0, 2)).astype(NPBF)
    woP = woP.reshape(128, 2 * C)

    CMP = SW * SD
    rq = 1.0 / np.sqrt(vq)
    rk = 1.0 / np.sqrt(vk)
    stats = np.stack([
        (rq / CMP).reshape(128, 128).T, (-mq * rq).reshape(128, 128).T,
        (rk / CMP).reshape(128, 128).T, (-mk * rk).reshape(128, 128).T,
    ], axis=1)          # [128 (s%128), 4, 128 (s//128)]
    stats = np.ascontiguousarray(stats.reshape(128, 4 * 128)).astype(np.float32)

    xs, xbs = [], []
    for b in range(B):
        xb = np.ascontiguousarray(
            x[b].reshape(S, 2, 128).transpose(2, 1, 0))   # [128, 2, S]
        xs.append(xb.astype(NPF8))
        xbs.append(xb.astype(NPBF))
    return xs, xbs, wqP, wqV, dwp, wvc, woP, stats, temp


def kernel(x, w_qkv, w_dw, w_out, temperature, mean_q, var_q, mean_k, var_k):
    xs, xbs, wqP, wqV, dwp, wvc, woP, stats, temp = _host_prep(
        x, w_qkv, w_dw, w_out, temperature, mean_q, var_q, mean_k, var_k)
    in_maps = []
    for b in range(B):
        in_maps.append({
            "xp": xs[b], "xb": xbs[b], "wqp": wqP, "wqv": wqV, "dwp": dwp,
            "wvcols": wvc, "wop": woP, "stats": stats, "temperature": temp,
        })
    global _last_in_maps
    _last_in_maps = in_maps
    nc = _get_nc()
    res = run_bass_kernel_spmd(nc, in_maps, list(range(B)))
    out = np.stack([np.asarray(res.results[b]["out"]).astype(np.float32)
                    for b in range(B)])
    return out.reshape(B, H, W, C)
